# revision 34
# baseline (speedup 1.0000x reference)
"""Low-rank (CPD) 3D conv kernel for Trainium2, SPMD across 8 NeuronCores.

Math (per reference):
  y[r,h,w,d]  = sum_c U_c_in[c,r] * x[c,h,w,d]
  y           = conv_h(conv_w(conv_d-separable 3-tap, per-rank taps U_k*))
  out[c,...]  = sum_r U_c_out[r,c] * z[r,...] + bias[c]

Distribution: data-parallel split of H (64) into 8 slabs of 8 planes; each
core reads its slab plus one halo plane on each side (zero at global edges)
and computes its output slab independently. No collectives.

Per-core pipeline, software-pipelined with a 2-plane skew so the in-order
PE queue never waits on the conv chain:
  iter i: mm1(i) -> mm1 drains(i) -> mm2(i-2) -> conv_w(i-1) -> conv_d(i-1)
Engines: mm1 folds conv_h (3 host-precomputed weight matrices per c-tile);
ScalarE does all PSUM drains plus the conv_d a2 scaled copy; VectorE does
the conv scaled copies (tensor_scalar, 4x mode) and shifted adds
(tensor_tensor, 2x mode, direct 2-byte-offset APs, no de-interleave);
GpSimd does only the final conv_d shifted add, off the critical path.
"""

import numpy as np
import ml_dtypes

BF16 = ml_dtypes.bfloat16

# Problem constants (hardcoded per contest contract)
C = 256   # input channels
R = 256   # rank
CO = 256  # output channels
S = 64    # spatial extent (cube)
NCORES = 8
HP = S // NCORES          # output planes per core (8)
HS = HP + 2               # slab planes incl. halo (10)
PLANE = S * S             # 4096 elements per (w,d) plane

_cache = {}


def _build_program(hp=HP):
    """Build and compile the per-core Bass program (identical on all cores)."""
    import concourse.bass as bass
    import concourse.mybir as mybir
    import concourse.tile as tile
    from concourse import bacc

    HS_ = hp + 2

    fp32 = mybir.dt.float32
    bf16 = mybir.dt.bfloat16

    nc = bacc.Bacc("TRN2", target_bir_lowering=False, debug=False,
                   num_devices=NCORES)

    x_d = nc.dram_tensor("xs", [2, 128, HS_, PLANE], bf16, kind="ExternalInput").ap()
    # all 12 mm1 weight matrices packed [128, 12*128]; 4 mm2 matrices [128, 4*128]
    wkh_d = nc.dram_tensor("wkh", [12, 128, 128], bf16, kind="ExternalInput").ap()
    uco_d = nc.dram_tensor("uco", [4, 128, 128], bf16, kind="ExternalInput").ap()
    # per-rank scalars packed: [2rt, 128, 7] = ukw(3) | ukd(3) | bias(1)
    scal_d = nc.dram_tensor("scal", [2, 128, 7], fp32, kind="ExternalInput").ap()
    out_d = nc.dram_tensor("out", [2, 128, hp, PLANE], fp32, kind="ExternalOutput").ap()

    add = mybir.AluOpType.add
    ident = mybir.ActivationFunctionType.Identity

    with tile.TileContext(nc) as tc:
        consts = tc.alloc_tile_pool(name="consts", bufs=1)
        xpool = tc.alloc_tile_pool(name="x", bufs=7)
        ypool = tc.alloc_tile_pool(name="y", bufs=4)
        tpool = tc.alloc_tile_pool(name="tmp", bufs=4)
        zpool = tc.alloc_tile_pool(name="z", bufs=2)
        zdpool = tc.alloc_tile_pool(name="zd", bufs=4)
        opool = tc.alloc_tile_pool(name="osb", bufs=2)
        ps1 = tc.alloc_tile_pool(name="ps1", bufs=2, space="PSUM")
        ps2 = tc.alloc_tile_pool(name="ps2", bufs=2, space="PSUM")

        # ---- constants: 4 batched DMAs (small, issued before the x bulk) ----
        wtile = consts.tile([128, 12 * 128], bf16, name="wkh", tag="wkh")
        nc.sync.dma_start(out=wtile.rearrange("p (m f) -> p m f", m=12),
                          in_=wkh_d.rearrange("m p f -> p m f"))
        wkh = [[[wtile[:, ((k * 2 + ct) * 2 + rt) * 128:
                       ((k * 2 + ct) * 2 + rt) * 128 + 128]
                 for rt in range(2)] for ct in range(2)] for k in range(3)]
        utile = consts.tile([128, 4 * 128], bf16, name="uco", tag="uco")
        nc.sync.dma_start(out=utile.rearrange("p (m f) -> p m f", m=4),
                          in_=uco_d.rearrange("m p f -> p m f"))
        uco = [[utile[:, (rt * 2 + co) * 128:(rt * 2 + co) * 128 + 128]
                for co in range(2)] for rt in range(2)]
        stile = [consts.tile([128, 7], fp32, name=f"scal{t}", tag=f"scal{t}")
                 for t in range(2)]
        for t in range(2):
            nc.sync.dma_start(out=stile[t], in_=scal_d[t])
        ukw = [stile[rt][:, 0:3] for rt in range(2)]
        ukd = [stile[rt][:, 3:6] for rt in range(2)]
        bia = [stile[co][:, 6:7] for co in range(2)]

        # ---- x plane streaming ----
        xt = {}

        def get_x(p, ct):
            if (p, ct) not in xt:
                t = xpool.tile([128, PLANE], bf16, name="xplane", tag="xplane")
                nc.sync.dma_start(out=t, in_=x_d[ct, :, p, :])
                xt[(p, ct)] = t
            return xt[(p, ct)]

        for p in range(3):
            for ct in range(2):
                get_x(p, ct)

        yt = {}     # mm1 outputs per (plane, rt)
        zt_ = {}    # conv_w outputs
        zdt = {}    # conv_d outputs

        def mm1(h):
            for rt in range(2):
                ysb = ypool.tile([128, PLANE], bf16, name="ysb", tag="y")
                if h == 0:
                    # k-outer ordering: the PE starts on x(0) before x(1)/x(2)
                    # have landed (head-of-pipeline DMA latency)
                    for hp2 in range(2):            # 2048-col half-planes
                        pts = [ps1.tile([128, 1024], fp32, name="pt1", tag="ps1")
                               for _ in range(2)]
                        for k in range(3):
                            for ct in range(2):
                                for q in range(2):
                                    for half in range(2):
                                        c0 = hp2 * 2048 + q * 1024 + half * 512
                                        nc.tensor.matmul(
                                            pts[q][:, half * 512:(half + 1) * 512],
                                            wkh[k][ct][rt],
                                            get_x(h + k, ct)[:, c0:c0 + 512],
                                            start=(k == 0 and ct == 0),
                                            stop=(k == 2 and ct == 1),
                                        )
                        for q in range(2):
                            dst = ysb[:, hp2 * 2048 + q * 1024:
                                      hp2 * 2048 + (q + 1) * 1024]
                            if rt == 0:
                                # drain plane 0 rt0 on DVE itself: moves the
                                # DVE pipeline start ~15us earlier (no ACT
                                # preamble / cross-engine wait)
                                nc.vector.tensor_scalar_mul(dst, pts[q], 1.0)
                            else:
                                nc.scalar.copy(dst, pts[q])
                else:
                    for q in range(PLANE // 1024):
                        pt = ps1.tile([128, 1024], fp32, name="pt1", tag="ps1")
                        for half in range(2):
                            first = True
                            for k in range(3):
                                for ct in range(2):
                                    nc.tensor.matmul(
                                        pt[:, half * 512:(half + 1) * 512],
                                        wkh[k][ct][rt],
                                        get_x(h + k, ct)[:, q * 1024 + half * 512:
                                                         q * 1024 + (half + 1) * 512],
                                        start=first,
                                        stop=(k == 2 and ct == 1),
                                    )
                                    first = False
                        nc.scalar.copy(ysb[:, q * 1024:(q + 1) * 1024], pt)
                yt[(h, rt)] = ysb

        def conv_w(h):
            for rt in range(2):
                y = yt.pop((h, rt))
                zt = zpool.tile([128, PLANE], bf16, name="zw", tag="z")
                t0 = tpool.tile([128, PLANE], bf16, name="t0t", tag="tmp")
                t2 = tpool.tile([128, PLANE], bf16, name="t2t", tag="tmp")
                nc.vector.tensor_scalar_mul(zt, y, ukw[rt][:, 1:2])
                nc.vector.tensor_scalar_mul(t0, y, ukw[rt][:, 0:1])
                nc.vector.tensor_scalar_mul(t2, y, ukw[rt][:, 2:3])
                zv = zt.rearrange("p (w d) -> p w d", d=64)
                t0v = t0.rearrange("p (w d) -> p w d", d=64)
                t2v = t2.rearrange("p (w d) -> p w d", d=64)
                nc.vector.tensor_tensor(zv[:, 1:, :], t0v[:, :-1, :], zv[:, 1:, :], add)
                nc.vector.tensor_tensor(zv[:, :-1, :], t2v[:, 1:, :], zv[:, :-1, :], add)
                zt_[(h, rt)] = zt

        avd = {}

        def conv_d_muls(h):
            # a0/a2 scaled copies: issued before the mm2 drains in the ACT
            # queue (those have a full plane of slack; these are needed by
            # DVE's shifted adds within this plane period)
            avs = []
            for rt in range(2):
                z = zt_[(h, rt)]
                a0 = tpool.tile([128, PLANE], bf16, name="a0t", tag="tmp")
                a2 = tpool.tile([128, PLANE], bf16, name="a2t", tag="tmp")
                # one of the four scaled copies on DVE keeps ACT's per-plane
                # work below the PE plane period (ACT is otherwise the pacer)
                if rt == 0:
                    nc.vector.tensor_scalar_mul(a0, z, ukd[rt][:, 0:1])
                else:
                    nc.scalar.mul(a0, z, ukd[rt][:, 0:1])
                nc.scalar.mul(a2, z, ukd[rt][:, 2:3])
                avs.append((a0, a2))
            avd[h] = avs

        def conv_d(h):
            avs = avd.pop(h)
            for rt in range(2):
                z = zt_.pop((h, rt))
                a0, a2 = avs[rt]
                zt = zdpool.tile([128, PLANE], bf16, name="zdt", tag="zd")
                nc.vector.tensor_scalar_mul(zt, z, ukd[rt][:, 1:2])
                zv = zt.rearrange("p (w d) -> p w d", d=64)
                a0v = a0.rearrange("p (w d) -> p w d", d=64)
                a2v = a2.rearrange("p (w d) -> p w d", d=64)
                nc.vector.tensor_tensor(zv[:, :, 1:], a0v[:, :, :-1], zv[:, :, 1:], add)
                nc.vector.tensor_tensor(zv[:, :, :-1], a2v[:, :, 1:], zv[:, :, :-1], add)
                zdt[(h, rt)] = zt

        def mm2(h):
            zd = [zdt.pop((h, 0)), zdt.pop((h, 1))]
            for co in range(2):
                osb = opool.tile([128, PLANE], fp32, name="osb", tag="osb")
                for q in range(PLANE // 1024):
                    pt = ps2.tile([128, 1024], fp32, name="pt2", tag="ps2")
                    for half in range(2):
                        for rt in range(2):
                            nc.tensor.matmul(
                                pt[:, half * 512:(half + 1) * 512],
                                uco[rt][co],
                                zd[rt][:, q * 1024 + half * 512:
                                       q * 1024 + (half + 1) * 512],
                                start=(rt == 0),
                                stop=(rt == 1),
                            )
                    nc.scalar.activation(osb[:, q * 1024:(q + 1) * 1024], pt,
                                         ident, bias=bia[co][:, 0:1])
                    if h == hp - 1 and q % 2 == 1:
                        # tail: overlap the final plane's store with its drains
                        nc.sync.dma_start(
                            out=out_d[co, :, h, (q - 1) * 1024:(q + 1) * 1024],
                            in_=osb[:, (q - 1) * 1024:(q + 1) * 1024])
                if h != hp - 1:
                    nc.sync.dma_start(out=out_d[co, :, h, :], in_=osb)

        # ---- software-pipelined schedule (2-plane skew for mm2) ----
        for i in range(hp + 2):
            if i <= hp - 1:
                mm1(i)
            if 1 <= i <= hp:
                conv_w(i - 1)
                conv_d_muls(i - 1)
            if 2 <= i <= hp + 1:
                mm2(i - 2)
            if 1 <= i <= hp:
                conv_d(i - 1)

        for pool in (ps2, ps1, opool, zdpool, zpool, tpool, ypool, xpool, consts):
            pool.release()

    nc.compile()
    return nc


def _host_prep(x, U_kh, U_kw, U_kd, U_c_in, U_c_out, bias):
    """Build per-core input maps (numpy only)."""
    x = np.asarray(x)
    U_kh = np.asarray(U_kh, np.float32)
    U_kw = np.asarray(U_kw, np.float32)
    U_kd = np.asarray(U_kd, np.float32)
    U_c_in = np.asarray(U_c_in, np.float32)
    U_c_out = np.asarray(U_c_out, np.float32)
    bias = np.asarray(bias, np.float32)

    xb = np.ascontiguousarray(x[0]).astype(BF16)          # [C, S, S, S]
    xb = xb.reshape(C, S, PLANE)

    # W_k[c, r] = U_c_in[c,r] * U_kh[k,r] packed [12, 128, 128] with
    # m = (k*2 + ct)*2 + rt
    wkh = np.empty((3, 2, 2, 128, 128), BF16)
    for k in range(3):
        wk = (U_c_in * U_kh[k][None, :]).astype(BF16)     # [C, R]
        wkh[k] = wk.reshape(2, 128, 2, 128).transpose(0, 2, 1, 3)
    wkh = np.ascontiguousarray(wkh.reshape(12, 128, 128))

    # U_c_out packed [4, 128, 128] with m = rt*2 + co
    uco = U_c_out.astype(BF16).reshape(2, 128, 2, 128).transpose(0, 2, 1, 3)
    uco = np.ascontiguousarray(uco.reshape(4, 128, 128))

    # per-rank/co scalars [2, 128, 7] = ukw | ukd | bias
    scal = np.empty((2, 128, 7), np.float32)
    scal[:, :, 0:3] = U_kw.T.reshape(2, 128, 3)
    scal[:, :, 3:6] = U_kd.T.reshape(2, 128, 3)
    scal[:, :, 6] = bias.reshape(2, 128)

    in_maps = []
    for c in range(NCORES):
        slab = np.zeros((C, HS, PLANE), BF16)
        lo, hi = c * HP - 1, c * HP + HP + 1
        s0, s1 = max(lo, 0), min(hi, S)
        slab[:, s0 - lo:HS - (hi - s1)] = xb[:, s0:s1]
        slab = np.ascontiguousarray(slab.reshape(2, 128, HS, PLANE))
        in_maps.append({
            "xs": slab, "wkh": wkh, "uco": uco, "scal": scal,
        })
    return in_maps


def kernel(x, U_kh, U_kw, U_kd, U_c_in, U_c_out, bias, _trace=False):
    from concourse.bass_utils import run_bass_kernel_spmd

    if "nc" not in _cache:
        _cache["nc"] = _build_program()
    nc = _cache["nc"]

    in_maps = _host_prep(x, U_kh, U_kw, U_kd, U_c_in, U_c_out, bias)
    res = run_bass_kernel_spmd(nc, in_maps, core_ids=list(range(NCORES)),
                               trace=_trace)
    _cache["last_result"] = res

    out = np.empty((1, CO, S, S, S), np.float32)
    for c in range(NCORES):
        o = res.results[c]["out"]                        # [2, 128, HP, PLANE]
        out[0, :, c * HP:(c + 1) * HP] = o.reshape(CO, HP, S, S)
    return out


# revision 35
# speedup vs baseline: 1.1636x; 1.1636x over previous
"""Low-rank (CPD) 3D conv kernel for Trainium2, SPMD across 8 NeuronCores.

Math (per reference):
  y[r,h,w,d]  = sum_c U_c_in[c,r] * x[c,h,w,d]
  y           = conv_h(conv_w(conv_d-separable 3-tap, per-rank taps U_k*))
  out[c,...]  = sum_r U_c_out[r,c] * z[r,...] + bias[c]

Distribution: data-parallel split of H (64) into 8 slabs of 8 planes; each
core reads its slab plus one halo plane on each side (zero at global edges)
and computes its output slab independently. No collectives.

Per-core pipeline, software-pipelined with a 2-plane skew so the in-order
PE queue never waits on the conv chain:
  iter i: mm1(i) -> mm1 drains(i) -> mm2(i-2) -> conv_w(i-1) -> conv_d(i-1)
Engines: mm1 folds conv_h (3 host-precomputed weight matrices per c-tile);
ScalarE does all PSUM drains plus the conv_d a2 scaled copy; VectorE does
the conv scaled copies (tensor_scalar, 4x mode) and shifted adds
(tensor_tensor, 2x mode, direct 2-byte-offset APs, no de-interleave);
GpSimd does only the final conv_d shifted add, off the critical path.
"""

import numpy as np
import ml_dtypes

BF16 = ml_dtypes.bfloat16

# Problem constants (hardcoded per contest contract)
C = 256   # input channels
R = 256   # rank
CO = 256  # output channels
S = 64    # spatial extent (cube)
NCORES = 8
HP = S // NCORES          # output planes per core (8)
HS = HP + 2               # slab planes incl. halo (10)
PLANE = S * S             # 4096 elements per (w,d) plane

_cache = {}


def _build_program(hp=HP):
    """Build and compile the per-core Bass program (identical on all cores)."""
    import concourse.bass as bass
    import concourse.mybir as mybir
    import concourse.tile as tile
    from concourse import bacc

    HS_ = hp + 2

    fp32 = mybir.dt.float32
    bf16 = mybir.dt.bfloat16

    nc = bacc.Bacc("TRN2", target_bir_lowering=False, debug=False,
                   num_devices=NCORES)

    x_d = nc.dram_tensor("xs", [2, 128, HS_, PLANE], bf16, kind="ExternalInput").ap()
    # all 12 mm1 weight matrices packed [128, 12*128]; 4 mm2 matrices [128, 4*128]
    wkh_d = nc.dram_tensor("wkh", [12, 128, 128], bf16, kind="ExternalInput").ap()
    uco_d = nc.dram_tensor("uco", [4, 128, 128], bf16, kind="ExternalInput").ap()
    # per-rank scalars packed: [2rt, 128, 7] = ukw(3) | ukd(3) | bias(1)
    scal_d = nc.dram_tensor("scal", [2, 128, 7], fp32, kind="ExternalInput").ap()
    out_d = nc.dram_tensor("out", [2, 128, hp, PLANE], fp32, kind="ExternalOutput").ap()

    add = mybir.AluOpType.add
    ident = mybir.ActivationFunctionType.Identity

    with tile.TileContext(nc) as tc:
        consts = tc.alloc_tile_pool(name="consts", bufs=1)
        xpool = tc.alloc_tile_pool(name="x", bufs=7)
        ypool = tc.alloc_tile_pool(name="y", bufs=4)
        tpool = tc.alloc_tile_pool(name="tmp", bufs=4)
        zpool = tc.alloc_tile_pool(name="z", bufs=2)
        zdpool = tc.alloc_tile_pool(name="zd", bufs=4)
        opool = tc.alloc_tile_pool(name="osb", bufs=2)
        ps1 = tc.alloc_tile_pool(name="ps1", bufs=2, space="PSUM")
        ps2 = tc.alloc_tile_pool(name="ps2", bufs=2, space="PSUM")

        # ---- constants: 4 batched DMAs (small, issued before the x bulk) ----
        wtile = consts.tile([128, 12 * 128], bf16, name="wkh", tag="wkh")
        nc.sync.dma_start(out=wtile.rearrange("p (m f) -> p m f", m=12),
                          in_=wkh_d.rearrange("m p f -> p m f"))
        wkh = [[[wtile[:, ((k * 2 + ct) * 2 + rt) * 128:
                       ((k * 2 + ct) * 2 + rt) * 128 + 128]
                 for rt in range(2)] for ct in range(2)] for k in range(3)]
        utile = consts.tile([128, 4 * 128], bf16, name="uco", tag="uco")
        nc.sync.dma_start(out=utile.rearrange("p (m f) -> p m f", m=4),
                          in_=uco_d.rearrange("m p f -> p m f"))
        uco = [[utile[:, (rt * 2 + co) * 128:(rt * 2 + co) * 128 + 128]
                for co in range(2)] for rt in range(2)]
        stile = [consts.tile([128, 7], fp32, name=f"scal{t}", tag=f"scal{t}")
                 for t in range(2)]
        for t in range(2):
            nc.sync.dma_start(out=stile[t], in_=scal_d[t])
        ukw = [stile[rt][:, 0:3] for rt in range(2)]
        ukd = [stile[rt][:, 3:6] for rt in range(2)]
        bia = [stile[co][:, 6:7] for co in range(2)]

        # ---- x plane streaming ----
        xt = {}

        def get_x(p, ct):
            if (p, ct) not in xt:
                t = xpool.tile([128, PLANE], bf16, name="xplane", tag="xplane")
                nc.sync.dma_start(out=t, in_=x_d[ct, :, p, :])
                xt[(p, ct)] = t
            return xt[(p, ct)]

        for p in range(3):
            for ct in range(2):
                get_x(p, ct)

        yt = {}     # mm1 outputs per (plane, rt)
        zt_ = {}    # conv_w outputs
        zdt = {}    # conv_d outputs

        def mm1(h):
            for rt in range(2):
                ysb = ypool.tile([128, PLANE], bf16, name="ysb", tag="y")
                if h == 0:
                    # k-outer ordering: the PE starts on x(0) before x(1)/x(2)
                    # have landed (head-of-pipeline DMA latency)
                    for hp2 in range(2):            # 2048-col half-planes
                        pts = [ps1.tile([128, 1024], fp32, name="pt1", tag="ps1")
                               for _ in range(2)]
                        for k in range(3):
                            for ct in range(2):
                                for q in range(2):
                                    for half in range(2):
                                        c0 = hp2 * 2048 + q * 1024 + half * 512
                                        nc.tensor.matmul(
                                            pts[q][:, half * 512:(half + 1) * 512],
                                            wkh[k][ct][rt],
                                            get_x(h + k, ct)[:, c0:c0 + 512],
                                            start=(k == 0 and ct == 0),
                                            stop=(k == 2 and ct == 1),
                                        )
                        for q in range(2):
                            dst = ysb[:, hp2 * 2048 + q * 1024:
                                      hp2 * 2048 + (q + 1) * 1024]
                            if rt == 0:
                                # drain plane 0 rt0 on DVE itself: moves the
                                # DVE pipeline start ~15us earlier (no ACT
                                # preamble / cross-engine wait)
                                nc.vector.tensor_scalar_mul(dst, pts[q], 1.0)
                            else:
                                nc.scalar.copy(dst, pts[q])
                else:
                    for q in range(PLANE // 1024):
                        pt = ps1.tile([128, 1024], fp32, name="pt1", tag="ps1")
                        for half in range(2):
                            first = True
                            for k in range(3):
                                for ct in range(2):
                                    nc.tensor.matmul(
                                        pt[:, half * 512:(half + 1) * 512],
                                        wkh[k][ct][rt],
                                        get_x(h + k, ct)[:, q * 1024 + half * 512:
                                                         q * 1024 + (half + 1) * 512],
                                        start=first,
                                        stop=(k == 2 and ct == 1),
                                    )
                                    first = False
                        nc.scalar.copy(ysb[:, q * 1024:(q + 1) * 1024], pt)
                yt[(h, rt)] = ysb

        def conv_w(h):
            for rt in range(2):
                y = yt.pop((h, rt))
                zt = zpool.tile([128, PLANE], bf16, name="zw", tag="z")
                t0 = tpool.tile([128, PLANE], bf16, name="t0t", tag="tmp")
                t2 = tpool.tile([128, PLANE], bf16, name="t2t", tag="tmp")
                nc.vector.tensor_scalar_mul(zt, y, ukw[rt][:, 1:2])
                nc.vector.tensor_scalar_mul(t0, y, ukw[rt][:, 0:1])
                nc.vector.tensor_scalar_mul(t2, y, ukw[rt][:, 2:3])
                zv = zt.rearrange("p (w d) -> p w d", d=64)
                t0v = t0.rearrange("p (w d) -> p w d", d=64)
                t2v = t2.rearrange("p (w d) -> p w d", d=64)
                nc.vector.tensor_tensor(zv[:, 1:, :], t0v[:, :-1, :], zv[:, 1:, :], add)
                nc.vector.tensor_tensor(zv[:, :-1, :], t2v[:, 1:, :], zv[:, :-1, :], add)
                zt_[(h, rt)] = zt

        def conv_d(h):
            # a0/a2 scaled copies on ScalarE (issued after this iter's PSUM
            # drains in the ACT queue, so they never block them)
            avs = []
            for rt in range(2):
                z = zt_[(h, rt)]
                a0 = tpool.tile([128, PLANE], bf16, name="a0t", tag="tmp")
                a2 = tpool.tile([128, PLANE], bf16, name="a2t", tag="tmp")
                # one of the four scaled copies on DVE keeps ACT's per-plane
                # work below the PE plane period (ACT is otherwise the pacer)
                if rt == 0:
                    nc.vector.tensor_scalar_mul(a0, z, ukd[rt][:, 0:1])
                else:
                    nc.scalar.mul(a0, z, ukd[rt][:, 0:1])
                nc.scalar.mul(a2, z, ukd[rt][:, 2:3])
                avs.append((a0, a2))
            for rt in range(2):
                z = zt_.pop((h, rt))
                a0, a2 = avs[rt]
                zt = zdpool.tile([128, PLANE], bf16, name="zdt", tag="zd")
                nc.vector.tensor_scalar_mul(zt, z, ukd[rt][:, 1:2])
                zv = zt.rearrange("p (w d) -> p w d", d=64)
                a0v = a0.rearrange("p (w d) -> p w d", d=64)
                a2v = a2.rearrange("p (w d) -> p w d", d=64)
                nc.vector.tensor_tensor(zv[:, :, 1:], a0v[:, :, :-1], zv[:, :, 1:], add)
                nc.vector.tensor_tensor(zv[:, :, :-1], a2v[:, :, 1:], zv[:, :, :-1], add)
                zdt[(h, rt)] = zt

        def mm2(h):
            zd = [zdt.pop((h, 0)), zdt.pop((h, 1))]
            for co in range(2):
                osb = opool.tile([128, PLANE], fp32, name="osb", tag="osb")
                for q in range(PLANE // 1024):
                    pt = ps2.tile([128, 1024], fp32, name="pt2", tag="ps2")
                    for half in range(2):
                        for rt in range(2):
                            nc.tensor.matmul(
                                pt[:, half * 512:(half + 1) * 512],
                                uco[rt][co],
                                zd[rt][:, q * 1024 + half * 512:
                                       q * 1024 + (half + 1) * 512],
                                start=(rt == 0),
                                stop=(rt == 1),
                            )
                    nc.scalar.activation(osb[:, q * 1024:(q + 1) * 1024], pt,
                                         ident, bias=bia[co][:, 0:1])
                    if h == hp - 1 and q % 2 == 1:
                        # tail: overlap the final plane's store with its drains
                        nc.sync.dma_start(
                            out=out_d[co, :, h, (q - 1) * 1024:(q + 1) * 1024],
                            in_=osb[:, (q - 1) * 1024:(q + 1) * 1024])
                if h != hp - 1:
                    nc.sync.dma_start(out=out_d[co, :, h, :], in_=osb)

        # ---- software-pipelined schedule (2-plane skew for mm2) ----
        for i in range(hp + 2):
            if i <= hp - 1:
                mm1(i)
            if 2 <= i <= hp + 1:
                mm2(i - 2)
            if 1 <= i <= hp:
                conv_w(i - 1)
                conv_d(i - 1)

        for pool in (ps2, ps1, opool, zdpool, zpool, tpool, ypool, xpool, consts):
            pool.release()

    nc.compile()
    return nc


def _host_prep(x, U_kh, U_kw, U_kd, U_c_in, U_c_out, bias):
    """Build per-core input maps (numpy only)."""
    x = np.asarray(x)
    U_kh = np.asarray(U_kh, np.float32)
    U_kw = np.asarray(U_kw, np.float32)
    U_kd = np.asarray(U_kd, np.float32)
    U_c_in = np.asarray(U_c_in, np.float32)
    U_c_out = np.asarray(U_c_out, np.float32)
    bias = np.asarray(bias, np.float32)

    xb = np.ascontiguousarray(x[0]).astype(BF16)          # [C, S, S, S]
    xb = xb.reshape(C, S, PLANE)

    # W_k[c, r] = U_c_in[c,r] * U_kh[k,r] packed [12, 128, 128] with
    # m = (k*2 + ct)*2 + rt
    wkh = np.empty((3, 2, 2, 128, 128), BF16)
    for k in range(3):
        wk = (U_c_in * U_kh[k][None, :]).astype(BF16)     # [C, R]
        wkh[k] = wk.reshape(2, 128, 2, 128).transpose(0, 2, 1, 3)
    wkh = np.ascontiguousarray(wkh.reshape(12, 128, 128))

    # U_c_out packed [4, 128, 128] with m = rt*2 + co
    uco = U_c_out.astype(BF16).reshape(2, 128, 2, 128).transpose(0, 2, 1, 3)
    uco = np.ascontiguousarray(uco.reshape(4, 128, 128))

    # per-rank/co scalars [2, 128, 7] = ukw | ukd | bias
    scal = np.empty((2, 128, 7), np.float32)
    scal[:, :, 0:3] = U_kw.T.reshape(2, 128, 3)
    scal[:, :, 3:6] = U_kd.T.reshape(2, 128, 3)
    scal[:, :, 6] = bias.reshape(2, 128)

    in_maps = []
    for c in range(NCORES):
        slab = np.zeros((C, HS, PLANE), BF16)
        lo, hi = c * HP - 1, c * HP + HP + 1
        s0, s1 = max(lo, 0), min(hi, S)
        slab[:, s0 - lo:HS - (hi - s1)] = xb[:, s0:s1]
        slab = np.ascontiguousarray(slab.reshape(2, 128, HS, PLANE))
        in_maps.append({
            "xs": slab, "wkh": wkh, "uco": uco, "scal": scal,
        })
    return in_maps


def kernel(x, U_kh, U_kw, U_kd, U_c_in, U_c_out, bias, _trace=False):
    from concourse.bass_utils import run_bass_kernel_spmd

    if "nc" not in _cache:
        _cache["nc"] = _build_program()
    nc = _cache["nc"]

    in_maps = _host_prep(x, U_kh, U_kw, U_kd, U_c_in, U_c_out, bias)
    res = run_bass_kernel_spmd(nc, in_maps, core_ids=list(range(NCORES)),
                               trace=_trace)
    _cache["last_result"] = res

    out = np.empty((1, CO, S, S, S), np.float32)
    for c in range(NCORES):
        o = res.results[c]["out"]                        # [2, 128, HP, PLANE]
        out[0, :, c * HP:(c + 1) * HP] = o.reshape(CO, HP, S, S)
    return out
